# revision 3
# baseline (speedup 1.0000x reference)
"""Adaptive Spatial Attention — batch-data-parallel across 8 NeuronCores.

Sharding: batch B=8 split 1-per-core (windows are independent, fully load
balanced); all parameters replicated. Tiny parameter transforms (pos-MLP ->
relative-position-bias tables, BN folding, weight transposes) are done once
on host; all heavy tensor work (qkv projections, window attention, depthwise
conv, spatial-interaction gating, output projection) runs on the NeuronCores.
"""
import numpy as np
import jax
import jax.numpy as jnp
from jax import lax

B, H, W, DIM, HEADS = 8, 128, 128, 192, 8
L = H * W
SPLIT = (4, 16)
HB = HEADS // 2          # heads per branch
CB = DIM // 2            # channels per branch
HD = CB // HB            # head dim = 24


# ---------------- host-side constant / parameter prep ----------------

def _make_rel(Hsp, Wsp):
    bh = np.arange(1 - Hsp, Hsp)
    bw = np.arange(1 - Wsp, Wsp)
    biases = np.stack(np.meshgrid(bh, bw, indexing='ij')).reshape(2, -1).T.astype(np.float32)
    coords = np.stack(np.meshgrid(np.arange(Hsp), np.arange(Wsp), indexing='ij')).reshape(2, -1)
    rel = (coords[:, :, None] - coords[:, None, :]).transpose(1, 2, 0).copy()
    rel[:, :, 0] += Hsp - 1
    rel[:, :, 1] += Wsp - 1
    rel[:, :, 0] *= 2 * Wsp - 1
    return biases, rel.sum(-1)


def _ln_np(x, g, b):
    x = x.astype(np.float32)
    m = x.mean(-1, keepdims=True)
    v = ((x - m) ** 2).mean(-1, keepdims=True)
    return (x - m) / np.sqrt(v + 1e-5) * g + b


def _dyn_bias_np(bi, pw, pb, g1, b1, w1, c1, g2, b2, w2, c2, g3, b3, w3, c3):
    p = bi @ pw.T + pb
    p = np.maximum(_ln_np(p, g1, b1), 0.0) @ w1.T + c1
    p = np.maximum(_ln_np(p, g2, b2), 0.0) @ w2.T + c2
    return np.maximum(_ln_np(p, g3, b3), 0.0) @ w3.T + c3  # (M, HB)


def _rpb_table(idx, pos_params):
    Hsp, Wsp = (SPLIT[0], SPLIT[1]) if idx == 0 else (SPLIT[1], SPLIT[0])
    N = Hsp * Wsp
    biases, rel = _make_rel(Hsp, Wsp)
    pos = _dyn_bias_np(biases, *[p[idx].astype(np.float32) for p in pos_params])
    rpb = pos[rel.reshape(-1)].reshape(N, N, HB).transpose(2, 0, 1)  # (HB, N, N)
    return np.ascontiguousarray(rpb.astype(np.float32))


# ---------------- device-side forward (one batch element) ----------------

def _branch(q, k, v, Hsp, Wsp, rpb):
    # q,k,v: (L, CB); rpb: (HB, N, N)
    N = Hsp * Wsp

    def win(t):  # (L, CB) -> (nW, HB, N, hd)
        t = t.reshape(H // Hsp, Hsp, W // Wsp, Wsp, CB).transpose(0, 2, 1, 3, 4)
        return t.reshape(-1, N, HB, HD).transpose(0, 2, 1, 3)

    qw, kw, vw = win(q), win(k), win(v)
    attn = jnp.einsum('whnd,whmd->whnm', qw * (HD ** -0.5), kw)
    attn = jax.nn.softmax(attn + rpb[None], axis=-1)
    z = jnp.einsum('whnm,whmd->whnd', attn, vw).transpose(0, 2, 1, 3).reshape(-1, N, CB)
    z = z.reshape(H // Hsp, W // Wsp, Hsp, Wsp, CB).transpose(0, 2, 1, 3, 4)
    return z.reshape(L, CB)


def _fwd1(x1, x2, wq, wk, wv, projT, proj_b, rpb0, rpb1, dw_w,
          sc1, sh1, si1T, si1_b, si2T, si2_b):
    # x1, x2: (L, C) one batch element. All weights pre-transposed/folded.
    q1 = x1 @ wq
    v1 = x1 @ wv
    k2 = x2 @ wk
    Ch = DIM // 2
    xa = _branch(q1[:, :Ch], k2[:, :Ch], v1[:, :Ch], SPLIT[0], SPLIT[1], rpb0)
    xb = _branch(q1[:, Ch:], k2[:, Ch:], v1[:, Ch:], SPLIT[1], SPLIT[0], rpb1)
    att = jnp.concatenate([xa, xb], axis=-1)  # (L, C)

    # depthwise 3x3 conv as 9 shifted multiply-adds in (H, W, C) layout
    # (avoids two 12.6MB transposes and the grouped-conv lowering)
    taps = dw_w[:, 0].transpose(1, 2, 0)                     # (3, 3, C)
    vp = jnp.pad(v1.reshape(H, W, DIM), ((1, 1), (1, 1), (0, 0)))
    acc = None
    for dr in range(3):
        for dc in range(3):
            t = vp[dr:dr + H, dc:dc + W, :] * taps[dr, dc][None, None, :]
            acc = t if acc is None else acc + t
    conv = acc.reshape(L, DIM) * sc1 + sh1                   # folded BN
    conv = jax.nn.gelu(conv, approximate=False)

    # spatial interaction gate from attention branch (1x1 -> BN -> GELU -> 1x1)
    s = att @ si1T + si1_b                                   # (L, 96), BN folded in
    s = jax.nn.gelu(s, approximate=False)
    s = s @ si2T + si2_b                                     # (L, 1)
    gate = jax.nn.sigmoid(s)                                 # (L, 1)

    out = (att + gate * conv) @ projT + proj_b
    return out


def kernel(x1, x2, qkv_w, proj_w, proj_b, pw, pb, g1, b1, w1, c1, g2, b2, w2, c2,
           g3, b3, w3, c3, dw_w, dw_b, bn1_g, bn1_b, bn1_m, bn1_v,
           si_w1, si_b1, bn2_g, bn2_b, bn2_m, bn2_v, si_w2, si_b2, H=None, W=None):
    f32 = np.float32
    # host parameter prep (tiny)
    wq = np.ascontiguousarray(qkv_w[0:DIM].T.astype(f32))          # (C, C)
    wk = np.ascontiguousarray(qkv_w[DIM:2 * DIM].T.astype(f32))
    wv = np.ascontiguousarray(qkv_w[2 * DIM:3 * DIM].T.astype(f32))
    projT = np.ascontiguousarray(proj_w.T.astype(f32))
    pos_params = (pw, pb, g1, b1, w1, c1, g2, b2, w2, c2, g3, b3, w3, c3)
    rpb0 = _rpb_table(0, pos_params)
    rpb1 = _rpb_table(1, pos_params)
    sc1 = (bn1_g / np.sqrt(bn1_v + 1e-5)).astype(f32)
    sh1 = ((dw_b - bn1_m) * sc1 + bn1_b).astype(f32)
    sc2 = (bn2_g / np.sqrt(bn2_v + 1e-5)).astype(f32)
    sh2 = ((si_b1 - bn2_m) * sc2 + bn2_b).astype(f32)
    si1T = np.ascontiguousarray((si_w1.T * sc2[None, :]).astype(f32))  # (C, 96) with BN scale folded
    si2T = np.ascontiguousarray(si_w2.T.astype(f32))               # (96, 1)

    fn = jax.pmap(_fwd1, in_axes=(0, 0) + (None,) * 14)
    out = fn(jnp.asarray(x1.astype(f32)), jnp.asarray(x2.astype(f32)),
             wq, wk, wv, projT, proj_b.astype(f32), rpb0, rpb1,
             dw_w.astype(f32), sc1, sh1, si1T, sh2,
             si2T, si_b2.astype(f32))
    return np.asarray(out).astype(f32)
